# revision 1
# baseline (speedup 1.0000x reference)
"""BiLSTM (B=128, T=256, H=512, L=2) Trainium2 Bass kernel.

Sharding: 8 cores = 2 directions x 4 batch-quarters (B_local=32).
Each core runs both layers of one direction on its batch shard; the two
layer-scans are wavefront-pipelined on-core. Host pre-flips time for the
backward-direction cores and re-flips + concatenates outputs.
"""

import numpy as np

import concourse.bacc as bacc
import concourse.mybir as mybir
import concourse.tile as tile
from concourse import bass_utils
from concourse.masks import make_identity

F32 = mybir.dt.float32
F32R = mybir.dt.float32r
AF = mybir.ActivationFunctionType
OP = mybir.AluOpType

B_FULL, T_FULL, H, L = 128, 256, 512, 2
G = 4 * H          # 2048
KT = H // 128      # 4 k-tiles
NT = G // 512      # 4 n-tiles (one per gate: i, f, g, o)
NCORES = 8
B_LOC = B_FULL // 4  # 32 per core


def ns(n):
    return slice(n * 512, (n + 1) * 512)


def build_bilstm(T=T_FULL, B=B_LOC, chunk=4, lag=8, mm_dt=mybir.dt.bfloat16, reps=1):
    assert B == 32 and T % chunk == 0 and lag > chunk
    nc = bacc.Bacc("TRN2", target_bir_lowering=False, debug=False)

    x = nc.dram_tensor("x", [B, T, H], F32, kind="ExternalInput").ap()
    wx0 = nc.dram_tensor("wx0", [H, G], F32, kind="ExternalInput").ap()
    wh0 = nc.dram_tensor("wh0", [H, G], F32, kind="ExternalInput").ap()
    wx1 = nc.dram_tensor("wx1", [H, G], F32, kind="ExternalInput").ap()
    wh1 = nc.dram_tensor("wh1", [H, G], F32, kind="ExternalInput").ap()
    b0r = nc.dram_tensor("b0r", [128, G], F32, kind="ExternalInput").ap()
    b1r = nc.dram_tensor("b1r", [128, G], F32, kind="ExternalInput").ap()
    out = nc.dram_tensor("out", [B, T, H], F32, kind="ExternalOutput").ap()

    def r(ap):
        return ap

    with tile.TileContext(nc) as tc:
        with (
            tc.tile_pool(name="dram", bufs=1, space="DRAM") as dram,
            tc.tile_pool(name="const", bufs=1) as const,
        ):
            xproj0 = dram.tile([T, B, G], mm_dt, name="xproj0")
            xproj1 = dram.tile([T, B, G], mm_dt, name="xproj1")

            ident = const.tile([128, 128], F32)
            make_identity(nc, ident)
            identr = const.tile([128, 128], mm_dt)
            nc.vector.tensor_copy(identr[:], ident[:])

            wh0_sb = const.tile([128, KT, G], mm_dt)
            wh1_sb = const.tile([128, KT, G], mm_dt)
            wx1_sb = const.tile([128, KT, G], mm_dt)
            b1r_sb = const.tile([128, G], F32)
            nc.gpsimd.dma_start(wh0_sb[:], wh0.rearrange("(kt p) n -> p kt n", p=128))
            nc.gpsimd.dma_start(wh1_sb[:], wh1.rearrange("(kt p) n -> p kt n", p=128))
            nc.gpsimd.dma_start(wx1_sb[:], wx1.rearrange("(kt p) n -> p kt n", p=128))
            nc.sync.dma_start(b1r_sb[:], b1r[:])

            for _rep in range(reps):
                # ---------------- Phase B: xproj0 = x @ wx0 + b0 -> DRAM ----------
                x_flat = x.rearrange("b t h -> (b t) h")
                n_mtiles = (B * T) // 128
                with (
                    tc.tile_pool(name="pb_w", bufs=1) as pbw,
                    tc.tile_pool(name="pb_x", bufs=3) as pbx,
                    tc.tile_pool(name="pb_xt", bufs=3) as pbxt,
                    tc.tile_pool(name="pb_o", bufs=3) as pbo,
                    tc.tile_pool(name="pb_pt", bufs=2, space="PSUM") as pbpt,
                    tc.tile_pool(name="pb_pg", bufs=6, space="PSUM") as pbpg,
                ):
                    wx0_sb = pbw.tile([128, KT, G], mm_dt)
                    b0r_sb = pbw.tile([128, G], F32)
                    nc.gpsimd.dma_start(wx0_sb[:], wx0.rearrange("(kt p) n -> p kt n", p=128))
                    nc.sync.dma_start(b0r_sb[:], b0r[:])

                    for m in range(n_mtiles):
                        xa = pbx.tile([128, H], F32, tag="xa")
                        nc.sync.dma_start(xa[:], x_flat[m * 128:(m + 1) * 128, :])
                        xt = pbxt.tile([128, KT, 128], mm_dt, tag="xt")
                        for kt in range(KT):
                            pt = pbpt.tile([128, 128], F32, tag="pt")
                            nc.tensor.transpose(
                                pt[:], xa[:, kt * 128:(kt + 1) * 128], ident[:]
                            )
                            nc.vector.tensor_copy(xt[:, kt, :], pt[:])
                        xo = pbo.tile([128, G], mm_dt, tag="xo")
                        for n in range(NT):
                            pg = pbpg.tile([128, 512], F32, tag="pg")
                            for kt in range(KT):
                                nc.tensor.matmul(
                                    pg[:],
                                    r(xt[:, kt, :]),
                                    r(wx0_sb[:, kt, ns(n)]),
                                    start=(kt == 0),
                                    stop=(kt == KT - 1),
                                )
                            nc.vector.tensor_tensor(
                                xo[:, ns(n)], pg[:], b0r_sb[:, ns(n)], op=OP.add
                            )
                        if T >= 128:
                            b_of = m // (T // 128)
                            th = m % (T // 128)
                            nc.sync.dma_start(
                                xproj0[th * 128:(th + 1) * 128, b_of, :], xo[:]
                            )
                        else:
                            nb = 128 // T
                            for bi in range(nb):
                                nc.sync.dma_start(
                                    xproj0[:, m * nb + bi, :], xo[bi * T:(bi + 1) * T, :]
                                )

                # ---------------- Phase C: the two wavefronted scans --------------
                mt_per_chunk = (chunk * 32) // 128  # 1 for chunk=4
                with (
                    tc.tile_pool(name="ring", bufs=2) as ring_pool,
                    tc.tile_pool(name="hcp", bufs=2) as hc_pool,
                    tc.tile_pool(name="xp0p", bufs=2) as xp0_pool,
                    tc.tile_pool(name="gp", bufs=2) as g_pool,
                    tc.tile_pool(name="mp", bufs=2) as m_pool,
                    tc.tile_pool(name="cp", bufs=2) as c_pool,
                    tc.tile_pool(name="hp", bufs=3) as h_pool,
                    tc.tile_pool(name="hTp", bufs=2) as hT_pool,
                    tc.tile_pool(name="pgps", bufs=5, space="PSUM") as pg_pool,
                    tc.tile_pool(name="ptps", bufs=1, space="PSUM") as pt_pool,
                    tc.tile_pool(name="pcps", bufs=2, space="PSUM") as pc_pool,
                ):
                    prev_c = c_pool.tile([64, 512], F32, tag="c")
                    nc.gpsimd.memset(prev_c[:], 0.0)
                    prev_hT = None
                    hc = None
                    ring_by_cidx = {}
                    gate_funcs = [AF.Sigmoid, AF.Sigmoid, AF.Tanh, AF.Sigmoid]

                    for tau in range(T + lag):
                        l0 = tau < T
                        l1 = tau >= lag
                        t0 = tau
                        t1 = tau - lag
                        p0 = 0 if l0 else 32
                        psz = (32 if l0 else 0) + (32 if l1 else 0)
                        sl = slice(p0, p0 + psz)

                        xp0 = xp0_pool.tile([64, G], mm_dt, tag="xp0")
                        if l0:
                            nc.sync.dma_start(xp0[0:32, :], xproj0[t0, :, :])
                        if l1:
                            nc.sync.dma_start(xp0[32:64, :], xproj1[t1, :, :])

                        # gate PSUM tiles + matmuls; n-outer so gates finish
                        # incrementally (order: i, g, f, o), kt-inner
                        pgs = [pg_pool.tile([64, 512], F32, tag="pg", name=f"pg_{tau}_{i}") for i in range(NT)]
                        gts = [
                            g_pool.tile([64, 512], F32, tag=f"g{n}", name=f"g{n}_{tau}")
                            for n in range(NT)
                        ]
                        for n in (0, 2, 1, 3):
                            for kt in range(KT):
                                if l0 and t0 > 0:
                                    nc.tensor.matmul(
                                        pgs[n][0:32, :],
                                        r(prev_hT[:, kt, 0:32]),
                                        r(wh0_sb[:, kt, ns(n)]),
                                        start=(kt == 0),
                                        stop=(kt == KT - 1),
                                        skip_group_check=True,
                                    )
                                if l1 and t1 > 0:
                                    nc.tensor.matmul(
                                        pgs[n][32:64, :],
                                        r(prev_hT[:, kt, 32:64]),
                                        r(wh1_sb[:, kt, ns(n)]),
                                        start=(kt == 0),
                                        stop=(kt == KT - 1),
                                        skip_group_check=True,
                                    )
                            ga = m_pool.tile(
                                [64, 512], F32, tag=f"ga{n}",
                                name=f"ga{n}_{tau}")
                            if l0 and l1 and t0 > 0 and t1 > 0:
                                nc.vector.tensor_tensor(
                                    ga[:, :], pgs[n][0:64, :],
                                    xp0[:, ns(n)], op=OP.add)
                                nc.scalar.activation(
                                    gts[n][0:64, :], ga[:, :], gate_funcs[n])
                            else:
                                if l0:
                                    if t0 > 0:
                                        nc.vector.tensor_tensor(
                                            ga[0:32, :], pgs[n][0:32, :],
                                            xp0[0:32, ns(n)], op=OP.add)
                                        nc.scalar.activation(
                                            gts[n][0:32, :], ga[0:32, :],
                                            gate_funcs[n])
                                    else:
                                        nc.scalar.activation(
                                            gts[n][0:32, :], xp0[0:32, ns(n)],
                                            gate_funcs[n])
                                if l1:
                                    if t1 > 0:
                                        nc.vector.tensor_tensor(
                                            ga[32:64, :], pgs[n][32:64, :],
                                            xp0[32:64, ns(n)], op=OP.add)
                                        nc.scalar.activation(
                                            gts[n][32:64, :], ga[32:64, :],
                                            gate_funcs[n])
                                    else:
                                        nc.scalar.activation(
                                            gts[n][32:64, :], xp0[32:64, ns(n)],
                                            gate_funcs[n])
                        g_i, g_f, g_g, g_o = gts

                        # cell update + h transpose, split into two free-dim
                        # halves to shorten the dependency tail
                        m1 = m_pool.tile([64, 512], F32, tag="m1")
                        m2 = m_pool.tile([64, 512], F32, tag="m2")
                        c_new = c_pool.tile([64, 512], F32, tag="c")
                        tch = m_pool.tile([64, 512], F32, tag="tc")
                        h_new = h_pool.tile([64, 512], F32, tag="h")
                        ptp = pt_pool.tile([128, KT, 64], F32, tag="ptp")
                        hT = hT_pool.tile([128, KT, 64], mm_dt, tag="hT")
                        if l0:
                            j0 = t0 % chunk
                            if j0 == 0:
                                hc = hc_pool.tile([128, KT, 32 * chunk], mm_dt, tag="hc")
                        for hi, hs in enumerate((0, 256)):
                            hsl = slice(hs, hs + 256)
                            nc.vector.tensor_tensor(
                                m1[sl, hsl], g_i[sl, hsl], g_g[sl, hsl], op=OP.mult)
                            nc.gpsimd.tensor_tensor(
                                m2[sl, hsl], g_f[sl, hsl], prev_c[sl, hsl], op=OP.mult)
                            nc.vector.tensor_tensor(
                                c_new[sl, hsl], m1[sl, hsl], m2[sl, hsl], op=OP.add)
                            nc.scalar.activation(
                                tch[sl, hsl], c_new[sl, hsl], AF.Tanh)
                            nc.gpsimd.tensor_tensor(
                                h_new[sl, hsl], g_o[sl, hsl], tch[sl, hsl], op=OP.mult)
                            kts = slice(2 * hi, 2 * hi + 2)
                            for kt in (2 * hi, 2 * hi + 1):
                                nc.tensor.transpose(
                                    ptp[:, kt, p0:p0 + psz],
                                    h_new[sl, kt * 128:(kt + 1) * 128],
                                    ident[sl, sl],
                                )
                            nc.vector.tensor_copy(
                                hT[:, kts, p0:p0 + psz], ptp[:, kts, p0:p0 + psz])
                            if l0:
                                nc.vector.tensor_copy(
                                    hc[:, kts, j0 * 32:(j0 + 1) * 32],
                                    ptp[:, kts, 0:32],
                                )
                        if tau == lag - 1:
                            nc.gpsimd.memset(c_new[32:64, :], 0.0)

                        # layer-1 output
                        if l1:
                            nc.sync.dma_start(out[:, t1, :], h_new[32:64, :])

                        # chunked layer-1 input projection GEMM on PE column
                        # strips 2-3 (M=64, psum partitions 64-127) so it runs
                        # concurrently with the scan matmuls on strips 0-1
                        if l0 and (t0 % chunk == chunk - 1):
                            cidx = t0 // chunk
                            mt_per_chunk = (chunk * 32) // 128
                            rt = ring_pool.tile([128, mt_per_chunk, G], mm_dt, tag="ring")
                            for mti in range(mt_per_chunk):
                                for n in range(NT):
                                    pc = pc_pool.tile([128, 512], F32, tag="pc")
                                    for kt in range(KT):
                                        nc.tensor.matmul(
                                            pc[:],
                                            r(hc[:, kt, mti * 128:(mti + 1) * 128]),
                                            r(wx1_sb[:, kt, ns(n)]),
                                            start=(kt == 0),
                                            stop=(kt == KT - 1),
                                        )
                                    nc.vector.tensor_tensor(
                                        rt[:, mti, ns(n)], pc[:],
                                        b1r_sb[:, ns(n)], op=OP.add)
                                nc.sync.dma_start(
                                    xproj1[cidx * chunk + mti * (128 // 32):
                                           cidx * chunk + (mti + 1) * (128 // 32),
                                           :, :],
                                    rt[:, mti, :])

                        prev_c = c_new
                        prev_hT = hT

    nc.compile()
    return nc


_NC_CACHE = {}


def _get_nc(T=T_FULL):
    if T not in _NC_CACHE:
        _NC_CACHE[T] = build_bilstm(T=T)
    return _NC_CACHE[T]


def _shard_inputs(x, Wx, Wh, b):
    """Build the 8 per-core input maps. Core c: direction d=c//4, shard s=c%4."""
    in_maps = []
    for c in range(NCORES):
        d, s = c // 4, c % 4
        xc = x[s * B_LOC:(s + 1) * B_LOC]
        if d == 1:
            xc = xc[:, ::-1, :]
        in_maps.append({
            "x": np.ascontiguousarray(xc, dtype=np.float32),
            "wx0": np.ascontiguousarray(Wx[0, d], dtype=np.float32),
            "wh0": np.ascontiguousarray(Wh[0, d], dtype=np.float32),
            "wx1": np.ascontiguousarray(Wx[1, d], dtype=np.float32),
            "wh1": np.ascontiguousarray(Wh[1, d], dtype=np.float32),
            "b0r": np.ascontiguousarray(
                np.broadcast_to(b[0, d], (128, G)), dtype=np.float32),
            "b1r": np.ascontiguousarray(
                np.broadcast_to(b[1, d], (128, G)), dtype=np.float32),
        })
    return in_maps


def _assemble(results):
    out = np.empty((B_FULL, T_FULL, 2 * H), dtype=np.float32)
    for c in range(NCORES):
        d, s = c // 4, c % 4
        oc = results[c]["out"]
        if d == 1:
            oc = oc[:, ::-1, :]
        out[s * B_LOC:(s + 1) * B_LOC, :, d * H:(d + 1) * H] = oc
    return out


def run_kernel(x, Wx, Wh, b, trace=False):
    nc = _get_nc()
    in_maps = _shard_inputs(
        np.asarray(x), np.asarray(Wx), np.asarray(Wh), np.asarray(b)
    )
    res = bass_utils.run_bass_kernel_spmd(
        nc, in_maps, core_ids=list(range(NCORES)), trace=trace
    )
    return _assemble(res.results), res


def kernel(x, Wx, Wh, b):
    out, _ = run_kernel(x, Wx, Wh, b)
    return out

